# revision 45
# baseline (speedup 1.0000x reference)
"""Trainium2 Bass kernel for RealVirtualAttention (masked segment-mean pool + HAN
semantic attention), SPMD across 8 NeuronCores.

v2 strategy (bf16 streaming, balanced exact tiling):
  - 4096 graphs -> 64 blocks of 64 graphs cut at graph boundaries (batch is
    sorted). Blocks are sorted by node count and dealt into 8 slots x 8 cores
    so every core gets one block per slot and slots have near-equal tile
    counts; the compiled program pads only to the per-slot max (983 tiles vs
    1024 padded, and all cores finish together).
  - Node features are cast to bf16 on the host (rms error ~2e-3, well under
    the 2e-2 gate): halves HBM traffic and enables fast-weight-load on PE.
  - Per 128-node tile, a one-hot selector [128 nodes, 128 sel] is built from
    host-precomputed col = (batch - g_base) + 64*(z==VIRTUAL) via is_equal on
    DVE (every 4th tile on GpSimd to relieve the DVE sequencer), then
    matmul-accumulated into PSUM -> per-slot [128 = (graph, real/virt), 150]
    masked segment sums on TensorE.
  - Rows scaled by 1/max(count,1) -> means; tiny HAN head per slot; scores
    all-reduced (8 bytes) across cores; softmax beta; final combine is 3 wide
    matmuls producing res [64, 8*150], which the host scatters back to
    [B, 150] (free host-side reorder).
"""

import numpy as np

import concourse.bacc as bacc
import concourse.bass as bass
import concourse.tile as tile
import concourse.mybir as mybir
from concourse.bass_utils import run_bass_kernel_spmd

F32 = mybir.dt.float32
BF16 = mybir.dt.bfloat16
FP8 = mybir.dt.float8e4     # e4m3; used for the all-real node region
N_CORES = 8
B = 4096          # graphs
D = 150           # feature dim
A = 128           # attention hidden dim
GB = 64           # graphs per block (x2 metapaths = 128 PSUM rows)
NBLK = 8          # slots (blocks per core)
VIRTUAL_Z = 100
CH = 48           # max tiles per DMA chunk (~0.9MB fp8)
CHUNK_BUFS = 6    # deep prefetch: DMA has 45% slack; insulates HW jitter
OH_BUFS = 5       # slots per tag: 5 strips (40-tile lookahead) + 5 wide
OH_SPLIT = 4      # legacy knob (unused when OH_PATTERN is set)
OH_PATTERN = (0, 0, 1, 0, 1)   # 1 = build this tile's one-hot on GpSimd

_PROGRAM_CACHE: dict = {}
_SIM_MODE = False    # build 1-device program with the collective stubbed out
LAST_RESULTS = None  # BassKernelResults of the most recent run (for test.py)
LAST_NC = None       # compiled program of the most recent run (for test.py)
LAST_IN_MAPS = None  # per-core input maps of the most recent run (for test.py)


def _chunk_sizes(t):
    """Split t tiles into chunks of at most CH tiles."""
    out = []
    while t > 0:
        c = min(CH, t)
        # avoid a tiny trailing chunk: rebalance last two
        if 0 < t - c < 8 and c == CH:
            c = (t + 1) // 2
        out.append(c)
        t -= c
    return out


def _build_program(slot_tiles: tuple, real_tiles: tuple):
    """slot_tiles: per-slot tile counts (shared by all cores).
    real_tiles: per-slot count of leading tiles guaranteed all-real on every
    core (nodes reordered real-first on the host); those tiles use 64-wide
    one-hots (half the LDWEIGHTS cost, smaller DVE op). Tile 0 of each slot
    stays 128-wide so start=True initializes all 128 PSUM rows."""
    key = (slot_tiles, real_tiles)
    if key in _PROGRAM_CACHE:
        return _PROGRAM_CACHE[key]

    TOT = sum(slot_tiles)               # total tiles per core
    tails = [slot_tiles[j] - real_tiles[j] for j in range(NBLK)]
    R8 = sum(real_tiles)                # fp8-region tiles (all-real, no pad)
    R16 = TOT - R8                      # bf16 tail tiles (virtual + remainder)
    r8_off = np.concatenate([[0], np.cumsum(real_tiles)]).astype(int)
    r16_off = np.concatenate([[0], np.cumsum(tails)]).astype(int)
    # chunk layout: (slot, tile_offset_global, ntiles, kind, region_tile_off);
    # kind 8 = fp8 real region, 16 = bf16 tail (one chunk per slot). The
    # first chunk is small so the compute pipeline starts immediately.
    chunks = []
    for j in range(NBLK):
        slot_t0 = sum(slot_tiles[:j])
        rj = real_tiles[j]
        sizes = ([8] + _chunk_sizes(rj - 8) if j == 0 and rj > 16
                 else _chunk_sizes(rj))
        loc = 0
        for c in sizes:
            chunks.append((j, slot_t0 + loc, c, 8, r8_off[j] + loc))
            loc += c
        assert loc == rj
        chunks.append((j, slot_t0 + rj, tails[j], 16, r16_off[j]))

    nc = bacc.Bacc("TRN2", target_bir_lowering=False, debug=False,
                   num_devices=1 if _SIM_MODE else N_CORES)
    xdat8 = nc.declare_dram_parameter("xdat8", [128, R8 * D], FP8,
                                      isOutput=False)
    xdat16 = nc.declare_dram_parameter("xdat16", [128, R16 * D], BF16,
                                       isOutput=False)
    colp = nc.declare_dram_parameter("col", [128, TOT], F32, isOutput=False)
    iotap = nc.declare_dram_parameter("iota", [128, 128], BF16, isOutput=False)
    # all small f32 constants packed into one blob: a single HWDGE issue
    # (~630ns each) instead of six -- trims the startup serialization
    # layout: ident [0:128] | w1a [128:256] | b1 [256] | q [257] |
    #         scales [258:266] | w1b rows 0:22 [266:394]
    BLOBW = 394
    blobp = nc.declare_dram_parameter("blob", [128, BLOBW], F32, isOutput=False)
    resp = nc.declare_dram_parameter("res", [GB, NBLK * D], BF16, isOutput=True)

    CHF_MAX = CH * D

    with tile.TileContext(nc) as tc:
        with tc.tile_pool(name="const", bufs=1) as cpool, \
             tc.tile_pool(name="chunks", bufs=CHUNK_BUFS) as chpool, \
             tc.tile_pool(name="oh", bufs=OH_BUFS) as ohpool, \
             tc.tile_pool(name="small", bufs=1) as spool, \
             tc.tile_pool(name="xt", bufs=2) as xtpool, \
             tc.tile_pool(name="pm", bufs=2, space="PSUM") as pm, \
             tc.tile_pool(name="ptp", bufs=1, space="PSUM") as ptp, \
             tc.tile_pool(name="ph", bufs=1, space="PSUM") as ph, \
             tc.tile_pool(name="ps", bufs=1, space="PSUM") as ps, \
             tc.tile_pool(name="pout", bufs=2, space="PSUM") as pout, \
             tc.tile_pool(name="dram", bufs=1, space="DRAM") as dpool:

            TAILMAX = max(tails)

            def load_chunk(L, kind, roff):
                if kind == 8:
                    chk = chpool.tile([128, CHF_MAX], FP8, tag="chunk8")
                    nc.sync.dma_start(chk[:, 0:L * D],
                                      xdat8[:, roff * D:(roff + L) * D])
                else:
                    chk = chpool.tile([128, TAILMAX * D], BF16, tag="chunk16")
                    nc.sync.dma_start(chk[:, 0:L * D],
                                      xdat16[:, roff * D:(roff + L) * D])
                return chk

            # --- iota + col first: the first one-hot needs them; with fp8
            # chunks the DMA pool has slack, so this no longer delays the
            # stream and removes a ~12us PE startup stall ---
            # iota + col on the SYNC ring ahead of the chunk prefetches:
            # same-ring FIFO guarantees they transfer (and complete) first,
            # so the first one-hots aren't stuck behind 2MB of prefetch
            iota_t = cpool.tile([128, 128], BF16, tag="iota")
            nc.sync.dma_start(iota_t[:], iotap[:])
            col_t = cpool.tile([128, TOT], F32, tag="col")
            nc.sync.dma_start(col_t[:], colp[:])

            # --- prefetch the first chunks so streaming starts immediately ---
            prefetched = []
            for (j, t0, L, kind, roff) in chunks[:CHUNK_BUFS - 1]:
                prefetched.append(load_chunk(L, kind, roff))

            # --- remaining constants (first needed at slot 0's epilogue) ---
            blob_t = cpool.tile([128, BLOBW], F32, tag="blob")
            nc.scalar.dma_start(blob_t[:], blobp[:])
            means_all = cpool.tile([128, NBLK * D], F32, tag="means")
            scores_acc = cpool.tile([1, 128], F32, tag="sacc")

            eq = mybir.AluOpType.is_equal
            mult = mybir.AluOpType.mult

            # pre-warm the Exp activation table during streaming so the real
            # HW table load isn't paid inside the post-collective tail
            warm = spool.tile([1, 2], F32, tag="warm")
            nc.vector.memset(warm[:], 0.0)
            nc.scalar.activation(warm[:], warm[:],
                                 mybir.ActivationFunctionType.Exp)

            # --- main streaming loop: masked segment sums per slot ---
            nchunks = len(chunks)
            pack_counter = [0]
            for ci, (j, t0, L, kind, roff) in enumerate(chunks):
                if ci < len(prefetched):
                    chunk = prefetched[ci]
                else:
                    chunk = load_chunk(L, kind, roff)
                slot_start = (ci == 0 or chunks[ci - 1][0] != j)
                slot_stop = (ci == nchunks - 1 or chunks[ci + 1][0] != j)
                if slot_start:
                    psum_means = pm.tile([128, D], F32, tag="pmeans")
                last_chunk = (ci == nchunks - 1)
                slot_t0 = sum(slot_tiles[:j])
                # narrow one-hots are packed 8-per-strip-tile: writes 2..8 of
                # a strip share its buffer-recycle wait, cutting the DVE/Pool
                # sequencer's per-op semaphore cost ~8x
                strip = None
                strip_fill = 0
                for t in range(L):
                    gt = t0 + t
                    ts_i = gt - slot_t0        # tile index within the slot
                    narrow = kind == 8 and ts_i >= 1
                    if narrow:
                        if strip is None or strip_fill == 8:
                            strip = ohpool.tile([128, 512], FP8, tag="ohs")
                            strip_fill = 0
                            pk = pack_counter[0]
                            pack_counter[0] += 1
                            strip_eng = (nc.gpsimd
                                         if (not last_chunk and
                                             OH_PATTERN[pk % len(OH_PATTERN)])
                                         else nc.vector)
                        oh = strip[:, strip_fill * 64:(strip_fill + 1) * 64]
                        strip_fill += 1
                        eng = strip_eng
                        W = 64
                    elif kind == 8:            # slot's first tile, fp8 wide
                        ohw = ohpool.tile([128, 128], FP8, tag="ohw8")
                        oh = ohw[:, 0:128]
                        eng = nc.vector
                        W = 128
                    else:                      # bf16 tail tile, wide
                        ohw = ohpool.tile([128, 128], BF16, tag="ohw")
                        oh = ohw[:, 0:128]
                        eng = nc.vector
                        W = 128
                    eng.tensor_scalar(out=oh, in0=iota_t[:, 0:W],
                                      scalar1=col_t[:, gt:gt + 1],
                                      scalar2=None, op0=eq)
                    nc.tensor.matmul(psum_means[0:W, :], oh,
                                     chunk[:, t * D:(t + 1) * D],
                                     start=(slot_start and t == 0),
                                     stop=(slot_stop and t == L - 1))

                if not slot_stop:
                    continue

                # --- slot epilogue: means + attention scores ---
                msl = means_all[:, j * D:(j + 1) * D]
                nc.vector.tensor_scalar(out=msl, in0=psum_means[:],
                                        scalar1=blob_t[:, 258 + j:259 + j],
                                        scalar2=None, op0=mult)
                tp = ptp.tile([128, 256], F32, tag="tp")
                nc.tensor.transpose(tp[:, 0:128], means_all[:, j * D:j * D + 128],
                                    blob_t[:, 0:128])
                nc.tensor.transpose(tp[0:22, 128:256],
                                    means_all[:, j * D + 128:j * D + 150],
                                    blob_t[:, 0:128])
                xt = xtpool.tile([128, 256], F32, tag="xt")
                nc.scalar.copy(xt[:, 0:128], tp[:, 0:128])
                nc.scalar.copy(xt[0:22, 128:256], tp[0:22, 128:256])
                ph_t = ph.tile([128, 128], F32, tag="h")
                nc.tensor.matmul(ph_t[:], blob_t[:, 128:256], xt[:, 0:128],
                                 start=True, stop=False)
                nc.tensor.matmul(ph_t[:], blob_t[0:D - 128, 266:394],
                                 xt[0:22, 128:256],
                                 start=False, stop=True)
                ht = xtpool.tile([128, 128], F32, tag="ht")
                nc.scalar.activation(ht[:], ph_t[:],
                                     mybir.ActivationFunctionType.Tanh,
                                     bias=blob_t[:, 256:257])
                ps_t = ps.tile([1, 128], F32, tag="s")
                nc.tensor.matmul(ps_t[:], blob_t[:, 257:258], ht[:],
                                 start=True, stop=True)
                if j == 0:
                    nc.vector.tensor_copy(scores_acc[:], ps_t[:])
                else:
                    nc.vector.tensor_add(scores_acc[:], scores_acc[:], ps_t[:])

            # --- global beta via 8-byte AllReduce + softmax ---
            s2 = spool.tile([1, 2], F32, tag="s2")
            nc.vector.reduce_sum(out=s2[0:1, 0:1], in_=scores_acc[0:1, 0:64],
                                 axis=mybir.AxisListType.X)
            nc.vector.reduce_sum(out=s2[0:1, 1:2], in_=scores_acc[0:1, 64:128],
                                 axis=mybir.AxisListType.X)
            cc_in = dpool.tile([1, 2], F32)
            cc_out = dpool.tile([1, 2], F32)
            nc.sync.dma_start(cc_in[:], s2[:])
            if _SIM_MODE:
                nc.gpsimd.dma_start(cc_out[:], cc_in[:])
            else:
                nc.gpsimd.collective_compute(
                    "AllReduce", mybir.AluOpType.add,
                    replica_groups=[list(range(N_CORES))],
                    ins=[cc_in.opt()], outs=[cc_out.opt()])
            sg = spool.tile([1, 2], F32, tag="sg")
            nc.sync.dma_start(sg[:], cc_out[:])
            e = spool.tile([1, 2], F32, tag="e")
            nc.scalar.activation(e[:], sg[:], mybir.ActivationFunctionType.Exp,
                                 scale=1.0 / B)
            esum = spool.tile([1, 1], F32, tag="esum")
            nc.vector.reduce_sum(out=esum[:], in_=e[:], axis=mybir.AxisListType.X)
            erec = spool.tile([1, 1], F32, tag="erec")
            nc.vector.reciprocal(erec[:], esum[:])
            beta = spool.tile([1, 2], F32, tag="beta")
            nc.vector.tensor_scalar(out=beta[:], in0=e[:],
                                    scalar1=erec[0:1, 0:1], scalar2=None,
                                    op0=mult)
            ones_t = spool.tile([1, 128], F32, tag="ones")
            nc.vector.memset(ones_t[:], 1.0)
            pbb = ps.tile([128, 2], F32, tag="bb")
            nc.tensor.matmul(pbb[:], ones_t[:], beta[:], start=True, stop=True)
            beta_bc = spool.tile([128, 2], F32, tag="bbc")
            nc.vector.tensor_copy(beta_bc[:], pbb[:])
            tmp1 = spool.tile([128, 64], F32, tag="tmp1")
            nc.vector.tensor_scalar(out=tmp1[:], in0=blob_t[:, 0:64],
                                    scalar1=beta_bc[:, 0:1], scalar2=None,
                                    op0=mult)
            tmp2 = spool.tile([128, 64], F32, tag="tmp2")
            nc.vector.tensor_scalar(out=tmp2[:], in0=blob_t[:, 64:128],
                                    scalar1=beta_bc[:, 1:2], scalar2=None,
                                    op0=mult)
            bsel = spool.tile([128, 64], F32, tag="bsel")
            nc.vector.tensor_add(bsel[:], tmp1[:], tmp2[:])

            # --- final combine: 3 wide matmuls over [128, 8*150] means ---
            # result is written in bf16 (host casts back to f32): halves the
            # per-execution output readback over the tunnel
            NF = NBLK * D
            osb = cpool.tile([64, NF], BF16, tag="osb")
            off = 0
            while off < NF:
                w = min(512, NF - off)
                po = pout.tile([64, 512], F32, tag="po")
                nc.tensor.matmul(po[:, 0:w], bsel[:], means_all[:, off:off + w],
                                 start=True, stop=True)
                nc.vector.tensor_copy(osb[:, off:off + w], po[:, 0:w])
                off += w
            # single output DMA: one HWDGE issue (~630ns) beats three
            # overlapped issues for this small (150KB) transfer
            nc.scalar.dma_start(resp[:], osb[:])

    nc.compile()
    _PROGRAM_CACHE[key] = nc
    return nc


def _to_bf16(a):
    """Round-to-nearest-even fp32 -> bf16, returned as ml_dtypes.bfloat16."""
    import ml_dtypes
    return a.astype(ml_dtypes.bfloat16)


def kernel(out, z, batch, W1, b1, q, num_graphs):
    global LAST_RESULTS, LAST_NC, LAST_IN_MAPS
    out = np.ascontiguousarray(np.asarray(out, dtype=np.float32))
    z = np.asarray(z).astype(np.int64)
    batch = np.asarray(batch).astype(np.int64)
    W1 = np.asarray(W1, dtype=np.float32)
    b1 = np.asarray(b1, dtype=np.float32)
    q = np.asarray(q, dtype=np.float32)
    assert int(num_graphs) == B
    N = out.shape[0]
    assert out.shape[1] == D and W1.shape == (D, A)

    # --- blocks of GB graphs, cut at graph boundaries ---
    cuts = np.searchsorted(batch, np.arange(0, B + 1, GB))
    nb = np.diff(cuts)                                   # nodes per block [64]
    tiles = np.ceil(nb / 128.0).astype(int)
    # balanced assignment: sort blocks by size desc; slot j gets ranks
    # [8j, 8j+8); core c takes the c-th block of each slot.
    order = np.argsort(-nb, kind="stable")
    slot_blocks = order.reshape(NBLK, N_CORES)           # [slot, core] -> block
    slot_tiles = tuple(int(tiles[slot_blocks[j]].max()) for j in range(NBLK))
    TOT = sum(slot_tiles)
    t_offsets = np.concatenate([[0], np.cumsum(slot_tiles)])

    # leading all-real tiles per slot (nodes are reordered real-first below):
    # min over cores so the bound holds for every core's block in that slot
    is_virt = (z == VIRTUAL_Z)
    nreal = np.array([(~is_virt[cuts[k]:cuts[k + 1]]).sum() for k in range(64)])
    real_tiles = tuple(min(int((nreal[slot_blocks[j]] // 128).min()),
                           slot_tiles[j] - 1)
                       for j in range(NBLK))

    # --- per-(graph, metapath) reciprocal counts ---
    keyv = 2 * batch + (z == VIRTUAL_Z)
    cnt = np.bincount(keyv, minlength=2 * B).reshape(B, 2).astype(np.float32)
    rcnt = 1.0 / np.maximum(cnt, 1.0)                    # [B, 2]
    nvirt = (z == VIRTUAL_Z).astype(np.float32)

    import ml_dtypes
    iota = np.tile(np.arange(128, dtype=np.float32), (128, 1))
    iota16 = iota.astype(ml_dtypes.bfloat16)
    # constant blob (see _build_program layout); scales filled per core
    blob_base = np.zeros((128, 394), dtype=np.float32)
    blob_base[:, 0:128] = np.eye(128, dtype=np.float32)
    blob_base[:, 128:256] = W1[:128]
    blob_base[:, 256] = b1
    blob_base[:, 257] = q.ravel()
    blob_base[0:D - 128, 266:394] = W1[128:]

    R8 = sum(real_tiles)
    R16 = TOT - R8
    r8_off = np.concatenate([[0], np.cumsum(real_tiles)]).astype(int)
    tails = [slot_tiles[j] - real_tiles[j] for j in range(NBLK)]
    r16_off = np.concatenate([[0], np.cumsum(tails)]).astype(int)

    in_maps = []
    core_slot_block = []                      # per core: block id per slot
    for core in range(N_CORES):
        xarr8 = np.zeros((128, R8 * D), dtype=ml_dtypes.float8_e4m3)
        xarr16 = np.zeros((128, R16 * D), dtype=ml_dtypes.bfloat16)
        colv = np.full((TOT, 128), -1.0, dtype=np.float32)
        sc = np.empty((128, NBLK), dtype=np.float32)
        blocks_c = []
        for j in range(NBLK):
            bk = int(slot_blocks[j, core])
            blocks_c.append(bk)
            lo, hi = int(cuts[bk]), int(cuts[bk + 1])
            nbr = hi - lo
            tj = slot_tiles[j]
            rj = real_tiles[j]
            # reorder block nodes real-first: the leading rj tiles are
            # guaranteed all-real (fp8 region); the tail holds the real
            # remainder + virtual nodes + padding (bf16 region)
            bv = is_virt[lo:hi]
            perm = np.concatenate([np.nonzero(~bv)[0], np.nonzero(bv)[0]])
            slab = np.zeros((tj * 128, D), dtype=np.float32)
            slab[:nbr] = out[lo:hi][perm]
            slab = slab.reshape(tj, 128, D).transpose(1, 0, 2)  # [128, tj, D]
            xarr8[:, r8_off[j] * D:(r8_off[j] + rj) * D] = (
                slab[:, :rj].reshape(128, rj * D))
            xarr16[:, r16_off[j] * D:(r16_off[j] + tj - rj) * D] = (
                slab[:, rj:].reshape(128, (tj - rj) * D))
            cv = ((batch[lo:hi] - bk * GB) + GB * nvirt[lo:hi]
                  ).astype(np.float32)[perm]
            cslab = np.full(tj * 128, -1.0, dtype=np.float32)
            cslab[:nbr] = cv
            colv[t_offsets[j]:t_offsets[j] + tj] = cslab.reshape(tj, 128)
            gids = bk * GB + np.arange(GB)
            sc[0:64, j] = rcnt[gids, 0]
            sc[64:128, j] = rcnt[gids, 1]
        blob = blob_base.copy()
        blob[:, 258:266] = sc
        in_maps.append({
            "xdat8": xarr8, "xdat16": xarr16,
            "col": np.ascontiguousarray(colv.T),
            "iota": iota16, "blob": blob,
        })
        core_slot_block.append(blocks_c)

    nc = _build_program(slot_tiles, real_tiles)
    LAST_NC, LAST_IN_MAPS = nc, in_maps
    res = run_bass_kernel_spmd(nc, in_maps, core_ids=list(range(N_CORES)))
    LAST_RESULTS = res

    outp = np.empty((B // GB * GB, D), dtype=np.float32)
    for core in range(N_CORES):
        r = np.asarray(res.results[core]["res"], dtype=np.float32)  # [64, 8*150]
        for j in range(NBLK):
            bk = core_slot_block[core][j]
            outp[bk * GB:(bk + 1) * GB, :] = r[:, j * D:(j + 1) * D]
    return outp.astype(np.float32)
